# revision 1
# baseline (speedup 1.0000x reference)
"""Trainium2 Bass kernel: single-head causal self-attention.

Problem: x[B=8, S=2048, D=1024], Wq/Wk/Wv[D, H=64], bq/bk/bv[H].
    q = x@Wq+bq; k = x@Wk+bk; v = x@Wv+bv
    out = softmax(causal(q k^T) / sqrt(H)) @ v

Sharding: batch -> 8 NeuronCores (data parallel, no collectives).

Per-core strategy (PE-roofline oriented; ~65 us vs 103.5 us baseline):
  - x shard is host-transposed to bf16 [D, S] chunks, all loaded on the
    sync DMA ring in J order (strict FIFO keeps J0 from sharing HBM
    bandwidth with later chunks); the first chunk is split into 256 KiB
    pieces so the first projection matmul gates on ~380 KiB total
  - all matmuls bf16: Wq|Wk packed [D,128] stationary; Wv carries an
    appended zero column whose bias is 1.0, so P @ [V|1] yields the
    softmax denominator as an extra output column
  - scores are built transposed: S^T [128 k, 512 q] = K_i Q^T in PSUM,
    exp (scale=1/8, bf16 out) over two PSUM banks at a time gives P^T
    tiles in SBUF = the moving operand of O^T = (V|1)^T P^T
  - causal structure exploited at 128-block granularity: diagonal-block
    score and PV matmuls skip the fully-masked q-range; the 128x128
    triangle is masked by a PE accumulate-matmul that adds -1e30 above
    the diagonal (stationary Tneg x identity) before exp — no separate
    mask pass, exp output is exact zeros there
  - K^T is moved to partitions 0:64 by a PE selector matmul + vector
    copy instead of an SBUF->SBUF DMA (deterministic latency)
  - software pipelining: PE order is proj_J -> out_{J-1} -> att_J so
    the out-phase vector work overlaps projection matmuls
  - O^T [65, 512] chunks are PE-transposed back (bf16), normalized by
    the reciprocal of the ones-column, and DMA'd out per q-chunk; the
    final chunk's cols [0,256) are processed one attention pair early
"""

import sys

sys.path.insert(0, "/opt/trn_rl_repo")

import numpy as np

B, S, D, H = 8, 2048, 1024, 64
N_CORES = 8
SQ = 512            # q chunk (PSUM bank / fp32 moving max)
NQ = S // SQ        # 4
ND = D // 128       # 8 contraction chunks for projections
H1 = H + 1          # V plus ones column
TRIM = True         # skip fully-masked q-ranges in diagonal blocks

_CACHE = {}


def _build_nc():
    import concourse.tile as tile
    from concourse import bacc, mybir

    f32 = mybir.dt.float32
    bf16 = mybir.dt.bfloat16
    AF = mybir.ActivationFunctionType

    nc = bacc.Bacc(None, target_bir_lowering=False)
    xTp = nc.dram_tensor("xTp", [NQ, 128, ND * SQ], bf16, kind="ExternalInput")
    wqk = nc.dram_tensor("wqk", [128, ND * 2 * H], bf16, kind="ExternalInput")
    # constpack cols: [bqk | bv1 | identb(128) | Tneg(128) | selK(64)
    #                  | wv1(520)]
    # Tneg[p, i] = -1e30 where p < i: the causal-mask accumulate operand
    # (adds -inf above the diagonal). selK[p, i] = (p == 64 + i): extracts
    # the K^T half of qkt down to partitions 0:64 via one PE matmul.
    CP_N = 2 + 128 + 128 + H + ND * H1
    cpk = nc.dram_tensor("cpk", [128, CP_N], bf16, kind="ExternalInput")
    out = nc.dram_tensor("out", [S, H], f32, kind="ExternalOutput")

    with tile.TileContext(nc) as tc:
        from contextlib import ExitStack

        with ExitStack() as ctx:
            const = ctx.enter_context(tc.tile_pool(name="const", bufs=1))
            sb = ctx.enter_context(tc.tile_pool(name="sb", bufs=1))
            pt_pool = ctx.enter_context(tc.tile_pool(name="pt", bufs=6))
            o_pool = ctx.enter_context(tc.tile_pool(name="o", bufs=2))
            ps = ctx.enter_context(tc.tile_pool(name="ps", bufs=1, space="PSUM"))

            wqk_sb = const.tile([128, ND * 2 * H], bf16)
            cpk_sb = const.tile([128, 2 + 128 + 128 + H + ND * H1], bf16)
            identb_sb = cpk_sb[:, 2:130]
            tneg_sb = cpk_sb[:, 130:258]
            selk_sb = cpk_sb[:, 258 : 258 + H]
            wv1_sb = cpk_sb[:, 258 + H :]
            # tensor_scalar needs f32 scalar operands: up-convert biases
            bf = const.tile([128, 2], f32)
            bqk_sb = bf[:, 0:1]
            bv1_sb = bf[:H1, 1:2]

            xt = {}          # J -> [128, ND*SQ] bf16 (c-chunk at cols c*SQ)
            QKT = {}         # J -> [128, SQ] bf16 (Q^T rows 0:64, K^T 64:128)
            KT0 = {}         # J -> [64, SQ] bf16 at base partition 0
            Vones = {}       # t -> [128, 4*66] bf16 ((V|1) rows, stride 66)
            VT1 = {}         # J -> [H1, SQ] bf16 (V^T plus ones row)

            # ---- DMA issue order defines per-queue FIFO order ----
            # sync: wqk first half, then ALL x in J order (strict FIFO
            #       per ring means J0 never shares bandwidth with later
            #       chunks; mixing queues starves the small-descriptor
            #       one at packet granularity), then out stores.
            # scalar: no DMAs (exp engine stays responsive)
            # gpsimd: wqk second half + constpack only
            with nc.named_scope("load"):
                for J in range(NQ):
                    t_x = sb.tile([128, ND * SQ], bf16, tag=f"x{J}")
                    xt[J] = t_x
                # wqk first half on the sync head (HWDGE, gates the
                # first matmul); second half slots after the first x piece
                # (needed at qk c4). gpsimd carries ONLY constpack so wv1
                # lands before the first vv matmul (SWDGE serializes
                # back-to-back transfers at ~2.3us each).
                nc.sync.dma_start(wqk_sb[:, : 4 * 2 * H], wqk[:, : 4 * 2 * H])
                nc.gpsimd.dma_start(cpk_sb[:], cpk[:, :])
                # first x piece rides the otherwise-unused scalar ring,
                # concurrent with wqkA (exp doesn't start until ~23us)
                nc.scalar.dma_start(xt[0][:, : 2 * SQ], xTp[0, :, : 2 * SQ])
                for c0 in range(2, ND, 2):
                    nc.sync.dma_start(
                        xt[0][:, c0 * SQ : (c0 + 2) * SQ],
                        xTp[0, :, c0 * SQ : (c0 + 2) * SQ],
                    )
                    if c0 == 2:
                        nc.sync.dma_start(
                            wqk_sb[:, 4 * 2 * H :], wqk[:, 4 * 2 * H :]
                        )
                for J in range(1, NQ):
                    nc.sync.dma_start(xt[J][:], xTp[J, :, :])
                nc.vector.tensor_copy(bf[:], cpk_sb[:, 0:2])

            def proj(J):
                with nc.named_scope(f"proj{J}"):
                    qk = ps.tile([128, SQ], f32, tag="proj", bufs=2)
                    for c in range(ND):
                        nc.tensor.matmul(
                            qk[:],
                            wqk_sb[:, c * 2 * H : (c + 1) * 2 * H],
                            xt[J][:, c * SQ : (c + 1) * SQ],
                            start=(c == 0),
                            stop=(c == ND - 1),
                        )
                    qkt = sb.tile([128, SQ], bf16, tag=f"qkt{J}")
                    # K half first: the ksel selector below depends only
                    # on it, so KT0 is ready ~2us earlier
                    nc.vector.tensor_scalar_add(
                        qkt[H:, :], qk[H:, :], bqk_sb[H:, :]
                    )
                    nc.vector.tensor_scalar_add(
                        qkt[:H, :], qk[:H, :], bqk_sb[:H, :]
                    )
                    QKT[J] = qkt

                    # K^T to partitions 0:64 via PE selector (avoids an
                    # SBUF->SBUF DMA); stationary/moving both base 64.
                    # Uses the "ot" bank, idle during projections, so vv
                    # keeps its own proj buffer (no bias-add stall).
                    ksel = ps.tile([H, SQ], f32, tag="ot", bufs=1)
                    nc.tensor.matmul(
                        ksel[:],
                        identb_sb[H:, H:],
                        qkt[H:, :],
                        start=True,
                        stop=True,
                    )
                    kt0 = sb.tile([H, SQ], bf16, tag=f"kt0{J}")
                    nc.scalar.activation(kt0[:], ksel[:], AF.Copy)
                    KT0[J] = kt0

                    vv = ps.tile([H1, SQ], f32, tag="proj", bufs=2)
                    for c in range(ND):
                        nc.tensor.matmul(
                            vv[:],
                            wv1_sb[:, c * H1 : (c + 1) * H1],
                            xt[J][:, c * SQ : (c + 1) * SQ],
                            start=(c == 0),
                            stop=(c == ND - 1),
                        )
                    vt1 = sb.tile([H1, SQ], bf16, tag=f"vt1{J}")
                    nc.vector.tensor_scalar_add(vt1[:], vv[:], bv1_sb[:])
                    VT1[J] = vt1

            def vtrans(J):
                # V~ = (V|1) in [s, h'] rows via PE transposes, all four
                # into one PSUM tile -> one SBUF copy per q-chunk
                # stride 66 keeps each bf16 PSUM output 4-byte aligned
                with nc.named_scope(f"vtr{J}"):
                    vt1 = VT1[J]
                    pst = ps.tile([128, 4 * 66], bf16, tag="vtr", bufs=1)
                    for tt in range(4):
                        nc.tensor.transpose(
                            pst[:, tt * 66 : tt * 66 + H1],
                            vt1[:, tt * 128 : (tt + 1) * 128],
                            identb_sb[:H1, :H1],
                        )
                    vo = sb.tile([128, 4 * 66], bf16, tag=f"vo{J}")
                    nc.vector.tensor_copy(
                        vo[:].rearrange("p (t u) -> p t u", t=4)[:, :, 0:H1],
                        pst[:].rearrange("p (t u) -> p t u", t=4)[:, :, 0:H1],
                    )
                    Vones[J] = vo

            OT = {}

            def att(J):
                with nc.named_scope(f"att{J}"):
                    ot = ps.tile([H1, SQ], f32, tag="ot", bufs=1)
                    OT[J] = ot
                    nhalf = 2 * (J + 1)   # pairs of k-chunks
                    for ii in range(nhalf):
                        diag = ii >= 2 * J
                        st = ps.tile([128, 2 * SQ], f32, tag="st", bufs=2)
                        for h2 in range(2):
                            i = 2 * ii + h2
                            r = i - 4 * J
                            q0 = 128 * r if (TRIM and diag and r > 0) else 0
                            nc.tensor.matmul(
                                st[:, h2 * SQ + q0 : (h2 + 1) * SQ],
                                KT0[i // 4][:, (i % 4) * 128 : (i % 4 + 1) * 128],
                                QKT[J][:H, q0:],
                                start=True,
                                stop=not diag,
                            )
                            if diag:
                                # causal mask folded into PSUM: add -1e30
                                # above the diagonal of the 128x128
                                # triangle block (exp then yields 0 there)
                                nc.tensor.matmul(
                                    st[:, h2 * SQ + 128 * r :
                                       h2 * SQ + 128 * (r + 1)],
                                    tneg_sb[:],
                                    identb_sb[:],
                                    start=False,
                                    stop=True,
                                )
                        pt = pt_pool.tile([128, 2 * SQ], bf16, tag="pt")
                        if TRIM and ii == 2 * J + 1:
                            # last diagonal pair: only 384 of 1024 cols are
                            # read by the trimmed PV matmuls below
                            nc.scalar.activation(
                                pt[:, 256:512], st[:, 256:512],
                                AF.Exp, scale=0.125,
                            )
                            nc.scalar.activation(
                                pt[:, 896:1024], st[:, 896:1024],
                                AF.Exp, scale=0.125,
                            )
                        else:
                            nc.scalar.activation(
                                pt[:], st[:], AF.Exp, scale=0.125
                            )
                        for h2 in range(2):
                            i = 2 * ii + h2
                            r = i - 4 * J
                            q0 = 128 * r if (TRIM and diag and r > 0) else 0
                            nc.tensor.matmul(
                                ot[:, q0:],
                                Vones[i // 4][:, (i % 4) * 66 : (i % 4) * 66 + H1],
                                pt[:, h2 * SQ + q0 : (h2 + 1) * SQ],
                                start=(i == 0),
                                stop=(i == 4 * (J + 1) - 1),
                            )

            def outp(J):
                # normalize + store rows 512J..512J+511. For the final
                # chunk, process cols [0,256) separately: they are final
                # one attention pair early (the last diagonal pair only
                # writes cols [256,512)), overlapping the tail.
                halves = [(0, 2), (2, 4)] if J == NQ - 1 else [(0, 4)]
                with nc.named_scope(f"out{J}"):
                    ot = OT[J]
                    ots = sb.tile([H1, SQ], bf16, tag=f"ots{J}")
                    ob = o_pool.tile([128, 4 * H], f32, tag="ob")
                    po = ps.tile([128, 4 * 66], bf16, tag="vtr", bufs=1)
                    rc = o_pool.tile([128, 4], f32, tag="rc")
                    for t0_, t1_ in halves:
                        # CAST + scaled copies ride the Activation engine
                        # (idle in proj/out windows) EXCEPT the very last
                        # half, which goes all-vector: scalar may still be
                        # draining the exp tail, and a single-engine chain
                        # avoids two cross-engine semaphore hops
                        last = J == NQ - 1
                        if last:
                            nc.vector.tensor_copy(
                                ots[:, t0_ * 128 : t1_ * 128],
                                ot[:, t0_ * 128 : t1_ * 128],
                            )
                        else:
                            nc.scalar.activation(
                                ots[:, t0_ * 128 : t1_ * 128],
                                ot[:, t0_ * 128 : t1_ * 128],
                                AF.Copy,
                            )
                        for tt in range(t0_, t1_):
                            nc.tensor.transpose(
                                po[:, tt * 66 : tt * 66 + H1],
                                ots[:, tt * 128 : (tt + 1) * 128],
                                identb_sb[:H1, :H1],
                            )
                        nc.vector.reciprocal(
                            rc[:, t0_:t1_],
                            po[:, t0_ * 66 + H : t1_ * 66 : 66],
                        )
                        for tt in range(t0_, t1_):
                            if last:
                                nc.vector.tensor_scalar_mul(
                                    ob[:, tt * H : (tt + 1) * H],
                                    po[:, tt * 66 : tt * 66 + H],
                                    rc[:, tt : tt + 1],
                                )
                            else:
                                nc.scalar.activation(
                                    ob[:, tt * H : (tt + 1) * H],
                                    po[:, tt * 66 : tt * 66 + H],
                                    AF.Copy,
                                    scale=rc[:, tt : tt + 1],
                                )
                        nc.sync.dma_start(
                            out[J * SQ + t0_ * 128 : J * SQ + t1_ * 128, :]
                            .rearrange("(t p) h -> p t h", p=128),
                            ob[:, t0_ * H : t1_ * H]
                            .rearrange("p (t h) -> p t h", t=t1_ - t0_),
                        )

            # software-pipelined schedule: out_{J-1} PE-transposes overlap
            # proj_J matmuls' vector work; att_J starts with kt0_J ready
            proj(0)
            vtrans(0)
            att(0)
            for J in range(1, NQ):
                proj(J)
                outp(J - 1)
                vtrans(J)
                att(J)
            outp(NQ - 1)

    nc.finalize()
    return nc


def _host_prep(x, Wq, bq, Wk, bk, Wv, bv):
    """Layout-only host prep: shard x by batch + pack weight operands."""
    import ml_dtypes

    f32 = np.float32
    bf16 = ml_dtypes.bfloat16
    wqk = np.concatenate([Wq, Wk], axis=1)          # [D, 128]
    # pack [D, M] -> [128, ND*M]: chunk c of 128 D-rows at cols c*M..
    wqk = np.ascontiguousarray(
        wqk.reshape(ND, 128, 2 * H).transpose(1, 0, 2).reshape(128, ND * 2 * H),
        dtype=bf16,
    )
    wv1 = np.concatenate([Wv, np.zeros((D, 1), f32)], axis=1)  # [D, 65]
    wv1 = np.ascontiguousarray(
        wv1.reshape(ND, 128, H1).transpose(1, 0, 2).reshape(128, ND * H1),
        dtype=bf16,
    )
    bqk = np.concatenate([bq, bk])[:, None].astype(bf16)       # [128, 1]
    bv1 = np.zeros((128, 1), bf16)
    bv1[:H1, 0] = np.concatenate([bv, np.ones((1,), f32)]).astype(bf16)
    # maskb[kk, v] = (v - 384 >= kk): block r's span mask (keep
    # qq >= 128r + kk over qq in [0, 128(r+1))) is maskb[:, 384-128r:512];
    # the shared 128x128 triangle is maskb[:, 384:512]
    kk = np.arange(128)[:, None]
    identb = np.eye(128, dtype=bf16)
    # Tneg[p, i] = -1e30 where p < i (adds -inf above the causal diagonal
    # when used as stationary with identity moving: out[i,j] += Tneg[j,i])
    tneg = np.where(kk < np.arange(128)[None, :], -1e30, 0.0).astype(bf16)
    # selK[p, i] = (p == 64 + i): extracts K^T half down to partitions 0:64
    selk = (kk == 64 + np.arange(H)[None, :]).astype(bf16)
    # constpack layout must match _build_nc:
    # [bqk | bv1 | identb | tneg | selk | wv1]
    cpk = np.ascontiguousarray(
        np.concatenate([bqk, bv1, identb, tneg, selk, wv1], axis=1),
        dtype=bf16,
    )
    common = {"wqk": wqk, "cpk": cpk}
    in_maps = []
    for b in range(B):
        m = dict(common)
        # xTp[J, p, c*SQ+s] = x[b][SQ*J+s, 128*c+p]
        m["xTp"] = np.ascontiguousarray(
            x[b].reshape(NQ, SQ, ND, 128).transpose(0, 3, 2, 1), dtype=bf16
        ).reshape(NQ, 128, ND * SQ)
        in_maps.append(m)
    return in_maps


def run(x, Wq, bq, Wk, bk, Wv, bv, trace=False):
    from concourse.bass_utils import run_bass_kernel_spmd

    if "nc" not in _CACHE:
        _CACHE["nc"] = _build_nc()
    nc = _CACHE["nc"]
    in_maps = _host_prep(
        np.asarray(x), np.asarray(Wq), np.asarray(bq), np.asarray(Wk),
        np.asarray(bk), np.asarray(Wv), np.asarray(bv),
    )
    res = run_bass_kernel_spmd(
        nc, in_maps, core_ids=list(range(N_CORES)), trace=trace
    )
    outs = np.stack([res.results[c]["out"] for c in range(N_CORES)], axis=0)
    return outs.astype(np.float32), res


def kernel(x, Wq, bq, Wk, bk, Wv, bv):
    outs, _ = run(x, Wq, bq, Wk, bk, Wv, bv, trace=False)
    return outs

